# revision 18
# baseline (speedup 1.0000x reference)
"""Trainium2 Bass kernel for nn_AnatomicalSpaceAttention_5952824672905.

Self-contained: kernel(**inputs) takes the FULL unsharded inputs, shards
across 8 NeuronCores (core c -> batch c//4, D-planes [8*(c%4), 8*(c%4)+8)),
runs one SPMD Bass graph (no collectives -- cross-attention rows are
independent), and gathers the full [B, C, D, H, W] f32 output.

v2:
- text branch (gelu/phase/k_rot/v: ~0.1% of FLOPs) folded into host prep,
  so the device does only the visual-side attention and ACT runs a single
  table set (exp) with zero reloads;
- 4-way PE tile packing for scores (row groups) and AV/den (col groups);
- N=256 q-projection matmuls;
- RoPE combine add offloaded to GPSIMD;
- bf16 output staging (upcast on host);
- cs/fv/wpack DMAs split across queues so the first exp fires ~10us in.
"""

import sys

for _p in ('/opt/trn_rl_repo', '/root/.axon_site/_ro/trn_rl_repo'):
    if _p not in sys.path:
        sys.path.append(_p)


import numpy as np
import ml_dtypes

import concourse.bass as bass
import concourse.mybir as mybir
import concourse.tile as tile
from concourse import bacc

BF16 = mybir.dt.bfloat16
F32 = mybir.dt.float32
AF = mybir.ActivationFunctionType

NH, HD = 8, 32
C, S, TD = 256, 256, 512
D = H = W = 32
N_CORES = 8
DSL = D // 4            # 8 d-planes per core
ROWS = DSL * H * W      # 8192
PT = 256                # rows per pair (one main-loop iteration)
NPAIRS = ROWS // PT     # 32
PPG = 8                 # pairs per fv DMA group
GR = PT * PPG           # 2048 rows per DMA group
NG = NPAIRS // PPG      # 4
OG = 4                  # pairs per output DMA group
SCALE = float(HD) ** -0.5
BASE = 10000.0

bf16 = ml_dtypes.bfloat16


# ----------------------------------------------------------------- host prep

def _inv_freq(dim):
    return 1.0 / (BASE ** (np.arange(0, dim, 2, dtype=np.float64) / dim))


def rope_freqs_full():
    """[D, H, W, HD] -- matches reference.rope3d_freqs."""
    zd = HD // 3
    yd = HD // 3
    xd = HD - zd - yd
    fz = np.arange(D, dtype=np.float64)[:, None] * _inv_freq(zd)   # [D, zd/2]
    fy = np.arange(H, dtype=np.float64)[:, None] * _inv_freq(yd)
    fx = np.arange(W, dtype=np.float64)[:, None] * _inv_freq(xd)
    ez = np.broadcast_to(np.concatenate([fz, fz], -1)[:, None, None, :], (D, H, W, zd))
    ey = np.broadcast_to(np.concatenate([fy, fy], -1)[None, :, None, :], (D, H, W, yd))
    ex = np.broadcast_to(np.concatenate([fx, fx], -1)[None, None, :, :], (D, H, W, xd))
    return np.concatenate([ez, ey, ex], axis=-1)  # [D,H,W,HD] f64


def rothalf_cols(a):
    """rotate_half applied per 32-channel head block along the last axis."""
    out = np.empty_like(a)
    for h in range(NH):
        b = a[:, h * HD:(h + 1) * HD]
        out[:, h * HD:h * HD + 16] = -b[:, 16:32]
        out[:, h * HD + 16:(h + 1) * HD] = b[:, 0:16]
    return out


def swap_w(wm):
    """Column-permute+negate so x @ w_sw == rotate_half(x @ w) per 32-head-dim."""
    w = np.asarray(wm)
    out = np.empty_like(w)
    for h in range(NH):
        blk = w[:, h * HD:(h + 1) * HD]
        out[:, h * HD:h * HD + 16] = -blk[:, 16:32]
        out[:, h * HD + 16:(h + 1) * HD] = blk[:, 0:16]
    return out


def _erf(x):
    try:
        from scipy.special import erf
        return erf(x)
    except Exception:
        import math
        return np.vectorize(math.erf)(x)


def gelu_exact(x):
    return 0.5 * x * (1.0 + _erf(x / np.sqrt(2.0)))


def host_prep(inputs):
    """Full inputs dict -> (in_maps list of 8 dicts).

    The text branch (h1 = gelu(text@m1), phase, k_rot, v) is S x TD per
    batch -- negligible FLOPs -- and is computed here in f64/f32 so the
    device graph is pure visual-side attention.
    """
    fv = np.asarray(inputs['fused_visual'], dtype=np.float32)   # [B,C,D,H,W]
    te = np.asarray(inputs['text_embedding'], dtype=np.float64)  # [B,S,TD]
    q_w = np.asarray(inputs['q_w'], dtype=np.float32)
    k_w = np.asarray(inputs['k_w'], dtype=np.float64)
    v_w = np.asarray(inputs['v_w'], dtype=np.float64)
    o_w = np.asarray(inputs['o_w'], dtype=np.float32)
    m1_w = np.asarray(inputs['m1_w'], dtype=np.float64)
    m2_w = np.asarray(inputs['m2_w'], dtype=np.float64)
    k_b = np.asarray(inputs['k_b'], dtype=np.float64)
    v_b = np.asarray(inputs['v_b'], dtype=np.float64)
    m1_b = np.asarray(inputs['m1_b'], dtype=np.float64)
    m2_b = np.asarray(inputs['m2_b'], dtype=np.float64)

    freqs = rope_freqs_full()                        # [D,H,W,HD]
    cosf = np.cos(freqs).astype(np.float32)
    sinf = np.sin(freqs).astype(np.float32)

    def chunks(w):
        w = np.asarray(w)
        return [w[i * 128:(i + 1) * 128, :] for i in range(w.shape[0] // 128)]

    # visual-side weights, identical across cores
    wvis = chunks(q_w * SCALE) + chunks(swap_w(q_w) * SCALE) + chunks(o_w)

    in_maps = []
    ktexts = {}
    for b in range(2):
        # text branch on host (f64)
        h1 = gelu_exact(te[b] @ m1_w + m1_b)             # [S, TD/2]
        phase = h1 @ m2_w + m2_b                         # [S, C]
        k = te[b] @ k_w + k_b                            # [S, C]
        krot = k * np.cos(phase) + rothalf_cols(k) * np.sin(phase)
        v = te[b] @ v_w + v_b                            # [S, C]
        krotT = np.ascontiguousarray(krot.T.astype(np.float32))  # [C, S]
        ktexts[b] = (chunks(krotT), chunks(v.astype(np.float32)))

    for c in range(N_CORES):
        b = c // 4
        g = c % 4
        dsl = slice(g * DSL, (g + 1) * DSL)
        fv_sh = np.ascontiguousarray(
            fv[b, :, dsl].reshape(C, ROWS)).astype(bf16)
        # cs [2, 128, ROWS]: cos/sin replicated 4x along partitions
        # (partition p = a*32 + j), row-major (d,h,w)
        cos_sh = np.tile(cosf[dsl].reshape(ROWS, HD).T, (4, 1))   # [128, ROWS]
        sin_sh = np.tile(sinf[dsl].reshape(ROWS, HD).T, (4, 1))
        cs = np.ascontiguousarray(
            np.stack([cos_sh, sin_sh], 0).astype(bf16))           # [2,128,ROWS]
        krot_chunks, v_chunks = ktexts[b]
        wall = np.ascontiguousarray(np.concatenate(
            wvis + krot_chunks + v_chunks, axis=1)).astype(bf16)  # [128, 10*256]
        m = {'fv': fv_sh, 'cs': cs, 'wpack': wall}
        in_maps.append(m)
    return in_maps


def gather_out(results):
    """Per-core [C, ROWS] bf16 -> full [B, C, D, H, W] f32."""
    B = 2
    out = np.empty((B, C, D, H, W), dtype=np.float32)
    for c in range(N_CORES):
        b = c // 4
        g = c % 4
        out[b, :, g * DSL:(g + 1) * DSL] = (
            results[c]['out'].astype(np.float32).reshape(C, DSL, H, W))
    return out


# ------------------------------------------------------------------- builder

SIM_SAFE = True


def build_nc(sim_safe=True):
    global SIM_SAFE
    SIM_SAFE = sim_safe
    nc = bacc.Bacc("TRN2", target_bir_lowering=False, debug=False)

    fv_d = nc.dram_tensor("fv", [C, ROWS], BF16, kind="ExternalInput")
    cs_d = nc.dram_tensor("cs", [2, 128, ROWS], BF16, kind="ExternalInput")
    wpack_d = nc.dram_tensor("wpack", [128, 10 * 256], BF16,
                             kind="ExternalInput")
    out_d = nc.dram_tensor("out", [C, ROWS], BF16, kind="ExternalOutput")

    with tile.TileContext(nc) as tc:
        _graph(tc, nc, fv_d, cs_d, wpack_d, out_d)

    nc.compile()
    return nc


def _graph(tc, nc, fv_d, cs_d, wpack_d, out_d):
    from contextlib import ExitStack
    ctx = ExitStack()
    with ctx:
        const = ctx.enter_context(tc.tile_pool(name="const", bufs=1))
        io = ctx.enter_context(tc.tile_pool(name="io", bufs=2))
        work = ctx.enter_context(tc.tile_pool(name="work", bufs=3))
        expp = ctx.enter_context(tc.tile_pool(name="expp", bufs=3))
        pq = ctx.enter_context(tc.tile_pool(name="pq", bufs=1, space="PSUM"))
        ps = ctx.enter_context(tc.tile_pool(name="ps", bufs=1, space="PSUM"))
        pa = ctx.enter_context(tc.tile_pool(name="pa", bufs=1, space="PSUM"))
        po = ctx.enter_context(tc.tile_pool(name="po", bufs=1, space="PSUM"))

        # PE warm-up burst first: no data deps, runs while DMAs stream, and
        # opens the HAM clock gate (1.2 -> 2.4 GHz) before real matmuls.
        # Sized to span the initial DMA wait (~9us): HAM flips to 8/8 after
        # ~3.4us of sustained PE activity and must not re-throttle before
        # the pair loop takes over. K=128 matmuls -- a K=1 burst does not
        # register as PE activity for the HAM monitor.
        wub = const.tile([128, 512], BF16)
        nc.vector.memset(wub, 0.0)
        wu = ps.tile([128, 4, PT], F32, tag="sp0", name="wu")
        wuf = wu.rearrange("p a r -> p (a r)")
        for i in range(18):
            nc.tensor.matmul(out=wuf[:, 0:512], lhsT=wub[:, 0:128],
                             rhs=wub, start=True, stop=True)

        # cs tables on the scalar-engine queue (group 0 first), everything
        # else on the sync queue: wpack, fv group 0, then prefetches + out.
        cs_sb = const.tile([128, 2, ROWS], BF16)

        def load_cs(gi, eng=None):
            # group 0 rides the scalar queue (free early); later groups go
            # on sync so their triggers don't delay the exp stream.
            eng = eng or nc.scalar
            for k in range(2):
                eng.dma_start(
                    out=cs_sb[:, k, gi * GR:(gi + 1) * GR],
                    in_=cs_d[k, :, gi * GR:(gi + 1) * GR])

        load_cs(0)

        wpack_sb = const.tile([128, 10, 256], BF16)
        nc.sync.dma_start(out=wpack_sb, in_=wpack_d.ap())

        _off = [0]

        def wview(kchunks):
            o = _off[0]
            _off[0] += kchunks
            return wpack_sb[:, o:o + kchunks, :]

        qw_sb = wview(2)
        qwsw_sb = wview(2)
        ow_sb = wview(2)
        krot_sb = wview(2)     # [128, 2(mc), S]: partition = channel mc*128+p
        v_sb = wview(2)        # [128, 2(sc), C]: partition = s pos sc*128+p

        fvst_tiles = {}

        def load_group(gi):
            t = io.tile([128, 2, GR], BF16, tag="fvst", name="fvst")
            for kc in range(2):
                nc.sync.dma_start(
                    out=t[:, kc, :],
                    in_=fv_d[kc * 128:(kc + 1) * 128, gi * GR:(gi + 1) * GR])
            fvst_tiles[gi] = t

        load_group(0)
        for gi in range(1, NG):
            load_cs(gi, eng=nc.sync)

        ones_sb = const.tile([128, HD], BF16)
        nc.vector.memset(ones_sb, 1.0)
        ones1_sb = const.tile([1, 128], BF16)
        nc.vector.memset(ones1_sb, 1.0)
        zeros512_sb = const.tile([1, 512], BF16)
        nc.vector.memset(zeros512_sb, 0.0)

        # ---------- main loop (baseline structure) ----------
        # Pair granularity (PT=256 rows). Scores/exp/attnv run in four
        # "quarters" per pair: qd = (half, g) with chunks (hp2, c) of
        # [128, 256]. Two 2-bank score slots ping-pong; av/den of quarter
        # qd-1 issue AFTER scores of quarter qd so ACT chains tightly.
        def qrope_phase(pi, mc):
            """q-proj + RoPE mul for pair pi, channel-chunk mc.

            PE: 4 MMs N=256 into a 1-bank qp; DVE: one FD=512 mul.
            The (cos, sin) combine add runs on GPSIMD at the call site.
            mc=1 borrows the o-proj bank (disjoint lifetime) so the mc=1
            fill doesn't stall on the mc=0 DVE mul draining the pq bank."""
            gi = pi // PPG
            p0 = (pi % PPG) * PT
            fvst = fvst_tiles[gi]
            pool, tg = (pq, "qp") if mc == 0 else (po, "op")
            qp = pool.tile([128, 2, PT], F32, tag=tg, name="qp")
            for sw in range(2):
                wsb = qw_sb if sw == 0 else qwsw_sb
                for kc in range(2):
                    nc.tensor.matmul(
                        out=qp[:, sw, :],
                        lhsT=wsb[:, kc, mc * 128:(mc + 1) * 128],
                        rhs=fvst[:, kc, p0:p0 + PT],
                        start=(kc == 0), stop=(kc == 1))
            tt = work.tile([128, 2, PT], F32, tag=f"tt{mc}", name="tt")
            nc.vector.tensor_mul(tt, qp, cs_sb[:, :, pi * PT:pi * PT + PT])
            return tt

        def quarter_scores(rot, qd):
            """Scores + exp for quarter qd=(half, g): 4 chunks (hp2, c)."""
            half, g = qd // 2, qd % 2
            sp = ps.tile([128, 2, 2, PT], F32, tag=f"sp{qd % 2}",
                         name="sp", uniquify=True)
            ex = expp.tile([128, 2, 2, PT], BF16, tag=f"ex{qd % 2}",
                           name="ex", uniquify=True)
            for c in range(2):
                for hp2 in range(2):
                    hp = 2 * half + hp2
                    nc.tensor.matmul(
                        out=sp[:, hp2, c, :],
                        lhsT=krot_sb[32 * hp:32 * hp + 32, g,
                                     c * 128:(c + 1) * 128],
                        rhs=rot[32 * hp:32 * hp + 32, g, :],
                        start=True, stop=True,
                        tile_position=(32 * hp, 0))
            nc.scalar.activation(out=ex, in_=sp, func=AF.Exp)
            return ex

        def quarter_avden(avd, ex, qd):
            # SIM_SAFE: groups opened/closed by opener/closer matmuls.
            # HW: stop is a sim-only concept; per-(bank, partition-set)
            # first_mm on the first write is what matters.
            half, g = qd // 2, qd % 2
            for c in range(2):
                for hp2 in range(2):
                    hp = 2 * half + hp2
                    h0 = 32 * (4 * g + hp)
                    nc.tensor.matmul(
                        out=avd[32 * hp:32 * hp + 32, g, :],
                        lhsT=v_sb[:, c, h0:h0 + 32],
                        rhs=ex[:, hp2, c, :],
                        start=(not SIM_SAFE and g == 0 and c == 0),
                        stop=(not SIM_SAFE and g == 1 and c == 1),
                        skip_group_check=not SIM_SAFE,
                        tile_position=(0, 32 * hp))
            for c in range(2):
                for hp2 in range(2):
                    hp = 2 * half + hp2
                    nc.tensor.matmul(
                        out=avd[32 * hp:32 * hp + 32, 2 + g, :],
                        lhsT=ones_sb,
                        rhs=ex[:, hp2, c, :],
                        start=(not SIM_SAFE and g == 0 and c == 0),
                        stop=(not SIM_SAFE and g == 1 and c == 1),
                        skip_group_check=not SIM_SAFE,
                        tile_position=(0, 32 * hp))

        # ---------- pair 0 prologue ----------
        tt0 = qrope_phase(0, 0)
        tt1 = qrope_phase(0, 1)
        rot_cur = work.tile([128, 2, PT], BF16, tag="rot", name="rot")
        nc.gpsimd.tensor_add(rot_cur[:, 0, :], tt0[:, 0, :], tt0[:, 1, :])
        nc.gpsimd.tensor_add(rot_cur[:, 1, :], tt1[:, 0, :], tt1[:, 1, :])

        outst = None
        epilogue = [None]
        for pi in range(NPAIRS):
            gi = pi // PPG
            if pi % PPG == 0 and gi + 1 < NG:
                load_group(gi + 1)
            if pi % OG == 0:
                outst = io.tile([128, 2, OG * PT], BF16, tag="outst",
                                name="outst")

            # avd {avA, avB | denA, denB} x PT rows = 2 banks; one zero-prime
            # opens each bank group, one zero-closer stops it.
            avd = pa.tile([128, 4, PT], F32, tag="avd", name="avd")
            avdf = avd.rearrange("p a r -> p (a r)")
            if SIM_SAFE:
                nc.tensor.matmul(out=avdf[:, 0:256], lhsT=ones1_sb,
                                 rhs=zeros512_sb[:, 0:256],
                                 start=True, stop=False)
                nc.tensor.matmul(out=avdf[:, 512:768], lhsT=ones1_sb,
                                 rhs=zeros512_sb[:, 0:256],
                                 start=True, stop=False)

            exq = [None] * 4
            rot_next = None
            tt_next = [None, None]
            for qd in range(5):
                if qd < 4:
                    exq[qd] = quarter_scores(rot_cur, qd)
                if qd == 0 and epilogue[0] is not None:
                    epilogue[0]()
                    epilogue[0] = None
                if qd == 1 and pi + 1 < NPAIRS:
                    tt_next[0] = qrope_phase(pi + 1, 0)
                if qd == 2 and pi + 1 < NPAIRS:
                    tt_next[1] = qrope_phase(pi + 1, 1)
                    rot_next = work.tile([128, 2, PT], BF16, tag="rot",
                                         name="rot")
                    nc.gpsimd.tensor_add(rot_next[:, 0, :],
                                         tt_next[0][:, 0, :],
                                         tt_next[0][:, 1, :])
                    nc.gpsimd.tensor_add(rot_next[:, 1, :],
                                         tt_next[1][:, 0, :],
                                         tt_next[1][:, 1, :])
                if qd > 0:
                    quarter_avden(avd, exq[qd - 1], qd - 1)
            if SIM_SAFE:
                nc.tensor.matmul(out=avdf[:, 0:512], lhsT=ones1_sb,
                                 rhs=zeros512_sb, start=False, stop=True)
                nc.tensor.matmul(out=avdf[:, 512:1024], lhsT=ones1_sb,
                                 rhs=zeros512_sb, start=False, stop=True)

            # recip + divide (pair level)
            rbc = work.tile([128, 2, PT], F32, tag="rbc", name="rbc")
            nc.vector.reciprocal_approx_fast(rbc, avd[:, 2:4, :])
            adiv = work.tile([128, 2, PT], BF16, tag="adiv", name="adiv")
            nc.vector.tensor_mul(adiv, avd[:, 0:2, :], rbc)

            # o-proj + stage-out, deferred into the next pair's first
            # quarter so the next exp chain isn't delayed
            def make_epilogue(adiv=adiv, pi=pi, outst=outst):
                def run():
                    op = po.tile([128, 2, PT], F32, tag="op", name="op")
                    for mc in range(2):
                        for g in range(2):
                            nc.tensor.matmul(
                                out=op[:, mc, :],
                                lhsT=ow_sb[:, g, mc * 128:(mc + 1) * 128],
                                rhs=adiv[:, g, :],
                                start=(g == 0), stop=(g == 1))
                    p0 = (pi % OG) * PT
                    nc.vector.tensor_copy(outst[:, :, p0:p0 + PT], op)
                    if pi % OG == OG - 1:
                        og0 = (pi // OG) * OG * PT
                        for mc in range(2):
                            nc.sync.dma_start(
                                out=out_d[mc * 128:(mc + 1) * 128,
                                          og0:og0 + OG * PT],
                                in_=outst[:, mc, :])
                return run

            epilogue[0] = make_epilogue()
            if pi == NPAIRS - 1:
                epilogue[0]()
                epilogue[0] = None

            if rot_next is not None:
                rot_cur = rot_next


_NC_CACHE = {}


def _get_nc():
    if 'nc' not in _NC_CACHE:
        _NC_CACHE['nc'] = build_nc(sim_safe=False)
    return _NC_CACHE['nc']


def _run(inputs, trace=False):
    from concourse.bass_utils import run_bass_kernel_spmd
    nc = _get_nc()
    in_maps = host_prep(inputs)
    res = run_bass_kernel_spmd(nc, in_maps, core_ids=list(range(N_CORES)),
                               trace=trace)
    return gather_out(res.results), res


def kernel(**inputs):
    out, _ = _run(inputs, trace=False)
    return out


# revision 20
# speedup vs baseline: 1.1817x; 1.1817x over previous
"""Trainium2 Bass kernel for nn_AnatomicalSpaceAttention_5952824672905.

Self-contained: kernel(**inputs) takes the FULL unsharded inputs, shards
across 8 NeuronCores (core c -> batch c//4, D-planes [8*(c%4), 8*(c%4)+8)),
runs one SPMD Bass graph (no collectives -- cross-attention rows are
independent), and gathers the full [B, C, D, H, W] f32 output.

v2:
- text branch (gelu/phase/k_rot/v: ~0.1% of FLOPs) folded into host prep,
  so the device does only the visual-side attention and ACT runs a single
  table set (exp) with zero reloads;
- 4-way PE tile packing for scores (row groups) and AV/den (col groups);
- N=256 q-projection matmuls;
- RoPE combine add offloaded to GPSIMD;
- bf16 output staging (upcast on host);
- cs/fv/wpack DMAs split across queues so the first exp fires ~10us in.
"""

import sys

for _p in ('/opt/trn_rl_repo', '/root/.axon_site/_ro/trn_rl_repo'):
    if _p not in sys.path:
        sys.path.append(_p)


import numpy as np
import ml_dtypes

import concourse.bass as bass
import concourse.mybir as mybir
import concourse.tile as tile
from concourse import bacc

BF16 = mybir.dt.bfloat16
F32 = mybir.dt.float32
AF = mybir.ActivationFunctionType

NH, HD = 8, 32
C, S, TD = 256, 256, 512
D = H = W = 32
N_CORES = 8
DSL = D // 4            # 8 d-planes per core
ROWS = DSL * H * W      # 8192
PT = 256                # rows per pair (one main-loop iteration)
NPAIRS = ROWS // PT     # 32
PPG = 8                 # pairs per fv DMA group
GR = PT * PPG           # 2048 rows per DMA group
NG = NPAIRS // PPG      # 4
OG = 4                  # pairs per output DMA group
SCALE = float(HD) ** -0.5
BASE = 10000.0

bf16 = ml_dtypes.bfloat16


# ----------------------------------------------------------------- host prep

def _inv_freq(dim):
    return 1.0 / (BASE ** (np.arange(0, dim, 2, dtype=np.float64) / dim))


def rope_freqs_full():
    """[D, H, W, HD] -- matches reference.rope3d_freqs."""
    zd = HD // 3
    yd = HD // 3
    xd = HD - zd - yd
    fz = np.arange(D, dtype=np.float64)[:, None] * _inv_freq(zd)   # [D, zd/2]
    fy = np.arange(H, dtype=np.float64)[:, None] * _inv_freq(yd)
    fx = np.arange(W, dtype=np.float64)[:, None] * _inv_freq(xd)
    ez = np.broadcast_to(np.concatenate([fz, fz], -1)[:, None, None, :], (D, H, W, zd))
    ey = np.broadcast_to(np.concatenate([fy, fy], -1)[None, :, None, :], (D, H, W, yd))
    ex = np.broadcast_to(np.concatenate([fx, fx], -1)[None, None, :, :], (D, H, W, xd))
    return np.concatenate([ez, ey, ex], axis=-1)  # [D,H,W,HD] f64


def rothalf_cols(a):
    """rotate_half applied per 32-channel head block along the last axis."""
    out = np.empty_like(a)
    for h in range(NH):
        b = a[:, h * HD:(h + 1) * HD]
        out[:, h * HD:h * HD + 16] = -b[:, 16:32]
        out[:, h * HD + 16:(h + 1) * HD] = b[:, 0:16]
    return out


def swap_w(wm):
    """Column-permute+negate so x @ w_sw == rotate_half(x @ w) per 32-head-dim."""
    w = np.asarray(wm)
    out = np.empty_like(w)
    for h in range(NH):
        blk = w[:, h * HD:(h + 1) * HD]
        out[:, h * HD:h * HD + 16] = -blk[:, 16:32]
        out[:, h * HD + 16:(h + 1) * HD] = blk[:, 0:16]
    return out


def _erf(x):
    try:
        from scipy.special import erf
        return erf(x)
    except Exception:
        import math
        return np.vectorize(math.erf)(x)


def gelu_exact(x):
    return 0.5 * x * (1.0 + _erf(x / np.sqrt(2.0)))


def host_prep(inputs):
    """Full inputs dict -> (in_maps list of 8 dicts).

    The text branch (h1 = gelu(text@m1), phase, k_rot, v) is S x TD per
    batch -- negligible FLOPs -- and is computed here in f64/f32 so the
    device graph is pure visual-side attention.
    """
    fv = np.asarray(inputs['fused_visual'], dtype=np.float32)   # [B,C,D,H,W]
    te = np.asarray(inputs['text_embedding'], dtype=np.float64)  # [B,S,TD]
    q_w = np.asarray(inputs['q_w'], dtype=np.float32)
    k_w = np.asarray(inputs['k_w'], dtype=np.float64)
    v_w = np.asarray(inputs['v_w'], dtype=np.float64)
    o_w = np.asarray(inputs['o_w'], dtype=np.float32)
    m1_w = np.asarray(inputs['m1_w'], dtype=np.float64)
    m2_w = np.asarray(inputs['m2_w'], dtype=np.float64)
    k_b = np.asarray(inputs['k_b'], dtype=np.float64)
    v_b = np.asarray(inputs['v_b'], dtype=np.float64)
    m1_b = np.asarray(inputs['m1_b'], dtype=np.float64)
    m2_b = np.asarray(inputs['m2_b'], dtype=np.float64)

    freqs = rope_freqs_full()                        # [D,H,W,HD]
    cosf = np.cos(freqs).astype(np.float32)
    sinf = np.sin(freqs).astype(np.float32)

    def chunks(w):
        w = np.asarray(w)
        return [w[i * 128:(i + 1) * 128, :] for i in range(w.shape[0] // 128)]

    # visual-side weights, identical across cores
    wvis = chunks(q_w * SCALE) + chunks(swap_w(q_w) * SCALE) + chunks(o_w)

    in_maps = []
    ktexts = {}
    for b in range(2):
        # text branch on host (f64)
        h1 = gelu_exact(te[b] @ m1_w + m1_b)             # [S, TD/2]
        phase = h1 @ m2_w + m2_b                         # [S, C]
        k = te[b] @ k_w + k_b                            # [S, C]
        krot = k * np.cos(phase) + rothalf_cols(k) * np.sin(phase)
        v = te[b] @ v_w + v_b                            # [S, C]
        krotT = np.ascontiguousarray(krot.T.astype(np.float32))  # [C, S]
        ktexts[b] = (chunks(krotT), chunks(v.astype(np.float32)))

    for c in range(N_CORES):
        b = c // 4
        g = c % 4
        dsl = slice(g * DSL, (g + 1) * DSL)
        fv_sh = np.ascontiguousarray(
            fv[b, :, dsl].reshape(C, ROWS)).astype(bf16)
        # cs [2, 128, ROWS]: cos/sin replicated 4x along partitions
        # (partition p = a*32 + j), row-major (d,h,w)
        cos_sh = np.tile(cosf[dsl].reshape(ROWS, HD).T, (4, 1))   # [128, ROWS]
        sin_sh = np.tile(sinf[dsl].reshape(ROWS, HD).T, (4, 1))
        cs = np.ascontiguousarray(
            np.stack([cos_sh, sin_sh], 0).astype(bf16))           # [2,128,ROWS]
        krot_chunks, v_chunks = ktexts[b]
        wall = np.ascontiguousarray(np.concatenate(
            wvis + krot_chunks + v_chunks, axis=1)).astype(bf16)  # [128, 10*256]
        m = {'fv': fv_sh, 'cs': cs, 'wpack': wall}
        in_maps.append(m)
    return in_maps


def gather_out(results):
    """Per-core [C, ROWS] bf16 -> full [B, C, D, H, W] f32."""
    B = 2
    out = np.empty((B, C, D, H, W), dtype=np.float32)
    for c in range(N_CORES):
        b = c // 4
        g = c % 4
        out[b, :, g * DSL:(g + 1) * DSL] = (
            results[c]['out'].astype(np.float32).reshape(C, DSL, H, W))
    return out


# ------------------------------------------------------------------- builder

SIM_SAFE = True


def build_nc(sim_safe=True):
    global SIM_SAFE
    SIM_SAFE = sim_safe
    nc = bacc.Bacc("TRN2", target_bir_lowering=False, debug=False)

    fv_d = nc.dram_tensor("fv", [C, ROWS], BF16, kind="ExternalInput")
    cs_d = nc.dram_tensor("cs", [2, 128, ROWS], BF16, kind="ExternalInput")
    wpack_d = nc.dram_tensor("wpack", [128, 10 * 256], BF16,
                             kind="ExternalInput")
    out_d = nc.dram_tensor("out", [C, ROWS], BF16, kind="ExternalOutput")

    with tile.TileContext(nc) as tc:
        _graph(tc, nc, fv_d, cs_d, wpack_d, out_d)

    nc.compile()
    return nc


def _graph(tc, nc, fv_d, cs_d, wpack_d, out_d):
    from contextlib import ExitStack
    ctx = ExitStack()
    with ctx:
        const = ctx.enter_context(tc.tile_pool(name="const", bufs=1))
        io = ctx.enter_context(tc.tile_pool(name="io", bufs=2))
        work = ctx.enter_context(tc.tile_pool(name="work", bufs=3))
        expp = ctx.enter_context(tc.tile_pool(name="expp", bufs=3))
        pq = ctx.enter_context(tc.tile_pool(name="pq", bufs=1, space="PSUM"))
        ps = ctx.enter_context(tc.tile_pool(name="ps", bufs=1, space="PSUM"))
        pa = ctx.enter_context(tc.tile_pool(name="pa", bufs=1, space="PSUM"))
        po = ctx.enter_context(tc.tile_pool(name="po", bufs=1, space="PSUM"))

        # PE warm-up burst first: no data deps, runs while DMAs stream, and
        # opens the HAM clock gate (1.2 -> 2.4 GHz) before real matmuls.
        # Sized to span the initial DMA wait (~9us): HAM flips to 8/8 after
        # ~3.4us of sustained PE activity and must not re-throttle before
        # the pair loop takes over. K=128 matmuls -- a K=1 burst does not
        # register as PE activity for the HAM monitor.
        wub = const.tile([128, 512], BF16)
        nc.vector.memset(wub, 0.0)
        wu = ps.tile([128, 4, PT], F32, tag="sp0", name="wu")
        wuf = wu.rearrange("p a r -> p (a r)")
        for i in range(18):
            nc.tensor.matmul(out=wuf[:, 0:512], lhsT=wub[:, 0:128],
                             rhs=wub, start=True, stop=True)

        # cs tables on the scalar-engine queue (group 0 first), everything
        # else on the sync queue: wpack, fv group 0, then prefetches + out.
        cs_sb = const.tile([128, 2, ROWS], BF16)

        def load_cs(gi, eng=None):
            # group 0 rides the scalar queue (free early); later groups go
            # on sync so their triggers don't delay the exp stream.
            eng = eng or nc.scalar
            for k in range(2):
                eng.dma_start(
                    out=cs_sb[:, k, gi * GR:(gi + 1) * GR],
                    in_=cs_d[k, :, gi * GR:(gi + 1) * GR])

        load_cs(0)

        wpack_sb = const.tile([128, 10, 256], BF16)
        nc.sync.dma_start(out=wpack_sb, in_=wpack_d.ap())

        _off = [0]

        def wview(kchunks):
            o = _off[0]
            _off[0] += kchunks
            return wpack_sb[:, o:o + kchunks, :]

        qw_sb = wview(2)
        qwsw_sb = wview(2)
        ow_sb = wview(2)
        krot_sb = wview(2)     # [128, 2(mc), S]: partition = channel mc*128+p
        v_sb = wview(2)        # [128, 2(sc), C]: partition = s pos sc*128+p

        fvst_tiles = {}

        def load_group(gi):
            t = io.tile([128, 2, GR], BF16, tag="fvst", name="fvst")
            for kc in range(2):
                nc.sync.dma_start(
                    out=t[:, kc, :],
                    in_=fv_d[kc * 128:(kc + 1) * 128, gi * GR:(gi + 1) * GR])
            fvst_tiles[gi] = t

        load_group(0)
        for gi in range(1, NG):
            load_cs(gi, eng=nc.sync)

        ones_sb = const.tile([128, HD], BF16)
        nc.vector.memset(ones_sb, 1.0)
        ones1_sb = const.tile([1, 128], BF16)
        nc.vector.memset(ones1_sb, 1.0)
        zeros512_sb = const.tile([1, 512], BF16)
        nc.vector.memset(zeros512_sb, 0.0)

        # ---------- main loop (baseline structure) ----------
        # Pair granularity (PT=256 rows). Scores/exp/attnv run in four
        # "quarters" per pair: qd = (half, g) with chunks (hp2, c) of
        # [128, 256]. Two 2-bank score slots ping-pong; av/den of quarter
        # qd-1 issue AFTER scores of quarter qd so ACT chains tightly.
        def qrope_phase(pi, mc):
            """q-proj + RoPE mul for pair pi, channel-chunk mc.

            PE: 4 MMs N=256 into a 1-bank qp; DVE: one FD=512 mul.
            The (cos, sin) combine add runs on GPSIMD at the call site.
            mc=1 borrows the o-proj bank (disjoint lifetime) so the mc=1
            fill doesn't stall on the mc=0 DVE mul draining the pq bank."""
            gi = pi // PPG
            p0 = (pi % PPG) * PT
            fvst = fvst_tiles[gi]
            pool, tg = (pq, "qp") if mc == 0 else (po, "op")
            qp = pool.tile([128, 2, PT], F32, tag=tg, name="qp")
            for sw in range(2):
                wsb = qw_sb if sw == 0 else qwsw_sb
                for kc in range(2):
                    nc.tensor.matmul(
                        out=qp[:, sw, :],
                        lhsT=wsb[:, kc, mc * 128:(mc + 1) * 128],
                        rhs=fvst[:, kc, p0:p0 + PT],
                        start=(kc == 0), stop=(kc == 1))
            tt = work.tile([128, 2, PT], F32, tag=f"tt{mc}", name="tt")
            nc.vector.tensor_mul(tt, qp, cs_sb[:, :, pi * PT:pi * PT + PT])
            return tt

        def quarter_scores(rot, qd):
            """Scores + exp for quarter qd=(half, g): 4 chunks (hp2, c)."""
            half, g = qd // 2, qd % 2
            sp = ps.tile([128, 2, 2, PT], F32, tag=f"sp{qd % 2}",
                         name="sp", uniquify=True)
            ex = expp.tile([128, 2, 2, PT], BF16, tag=f"ex{qd % 2}",
                           name="ex", uniquify=True)
            for c in range(2):
                for hp2 in range(2):
                    hp = 2 * half + hp2
                    nc.tensor.matmul(
                        out=sp[:, hp2, c, :],
                        lhsT=krot_sb[32 * hp:32 * hp + 32, g,
                                     c * 128:(c + 1) * 128],
                        rhs=rot[32 * hp:32 * hp + 32, g, :],
                        start=True, stop=True,
                        tile_position=(32 * hp, 0))
            nc.scalar.activation(out=ex, in_=sp, func=AF.Exp)
            return ex

        def quarter_avden(avd, ex, qd):
            # SIM_SAFE: groups opened/closed by opener/closer matmuls.
            # HW: stop is a sim-only concept; per-(bank, partition-set)
            # first_mm on the first write is what matters.
            half, g = qd // 2, qd % 2
            for c in range(2):
                for hp2 in range(2):
                    hp = 2 * half + hp2
                    h0 = 32 * (4 * g + hp)
                    nc.tensor.matmul(
                        out=avd[32 * hp:32 * hp + 32, g, :],
                        lhsT=v_sb[:, c, h0:h0 + 32],
                        rhs=ex[:, hp2, c, :],
                        start=(not SIM_SAFE and g == 0 and c == 0),
                        stop=(not SIM_SAFE and g == 1 and c == 1),
                        skip_group_check=not SIM_SAFE,
                        tile_position=(0, 32 * hp))
            for c in range(2):
                for hp2 in range(2):
                    hp = 2 * half + hp2
                    nc.tensor.matmul(
                        out=avd[32 * hp:32 * hp + 32, 2 + g, :],
                        lhsT=ones_sb,
                        rhs=ex[:, hp2, c, :],
                        start=(not SIM_SAFE and g == 0 and c == 0),
                        stop=(not SIM_SAFE and g == 1 and c == 1),
                        skip_group_check=not SIM_SAFE,
                        tile_position=(0, 32 * hp))

        # ---------- pair 0 prologue ----------
        tt0 = qrope_phase(0, 0)
        tt1 = qrope_phase(0, 1)
        rot_cur = work.tile([128, 2, PT], BF16, tag="rot", name="rot")
        nc.gpsimd.tensor_add(rot_cur[:, 0, :], tt0[:, 0, :], tt0[:, 1, :])
        nc.gpsimd.tensor_add(rot_cur[:, 1, :], tt1[:, 0, :], tt1[:, 1, :])

        outst = None
        epilogue = [None]
        for pi in range(NPAIRS):
            gi = pi // PPG
            if pi % PPG == 0 and gi + 1 < NG:
                load_group(gi + 1)
            if pi % OG == 0:
                outst = io.tile([128, 2, OG * PT], BF16, tag="outst",
                                name="outst")

            # avd {avA, avB | denA, denB} x PT rows = 2 banks; one zero-prime
            # opens each bank group, one zero-closer stops it.
            avd = pa.tile([128, 4, PT], F32, tag="avd", name="avd")
            avdf = avd.rearrange("p a r -> p (a r)")
            if SIM_SAFE:
                nc.tensor.matmul(out=avdf[:, 0:256], lhsT=ones1_sb,
                                 rhs=zeros512_sb[:, 0:256],
                                 start=True, stop=False)
                nc.tensor.matmul(out=avdf[:, 512:768], lhsT=ones1_sb,
                                 rhs=zeros512_sb[:, 0:256],
                                 start=True, stop=False)

            exq = [None] * 4
            rot_next = None
            tt_next = [None, None]
            for qd in range(5):
                if qd < 4:
                    exq[qd] = quarter_scores(rot_cur, qd)
                if qd == 0 and epilogue[0] is not None:
                    epilogue[0]()
                    epilogue[0] = None
                if qd == 1 and pi + 1 < NPAIRS:
                    tt_next[0] = qrope_phase(pi + 1, 0)
                if qd == 2 and pi + 1 < NPAIRS:
                    tt_next[1] = qrope_phase(pi + 1, 1)
                    rot_next = work.tile([128, 2, PT], BF16, tag="rot",
                                         name="rot")
                    nc.gpsimd.tensor_add(rot_next[:, 0, :],
                                         tt_next[0][:, 0, :],
                                         tt_next[0][:, 1, :])
                    nc.gpsimd.tensor_add(rot_next[:, 1, :],
                                         tt_next[1][:, 0, :],
                                         tt_next[1][:, 1, :])
                if qd > 0:
                    quarter_avden(avd, exq[qd - 1], qd - 1)
            if SIM_SAFE:
                nc.tensor.matmul(out=avdf[:, 0:512], lhsT=ones1_sb,
                                 rhs=zeros512_sb, start=False, stop=True)
                nc.tensor.matmul(out=avdf[:, 512:1024], lhsT=ones1_sb,
                                 rhs=zeros512_sb, start=False, stop=True)

            # recip + divide (pair level)
            rbc = work.tile([128, 2, PT], F32, tag="rbc", name="rbc")
            nc.vector.reciprocal_approx_fast(rbc, avd[:, 2:4, :])
            adiv = work.tile([128, 2, PT], BF16, tag="adiv", name="adiv")
            nc.vector.tensor_mul(adiv, avd[:, 0:2, :], rbc)

            # o-proj + stage-out, deferred into the next pair's first
            # quarter so the next exp chain isn't delayed
            def make_epilogue(adiv=adiv, pi=pi, outst=outst):
                def run():
                    op = po.tile([128, 2, PT], F32, tag="op", name="op")
                    for mc in range(2):
                        for g in range(2):
                            nc.tensor.matmul(
                                out=op[:, mc, :],
                                lhsT=ow_sb[:, g, mc * 128:(mc + 1) * 128],
                                rhs=adiv[:, g, :],
                                start=(g == 0), stop=(g == 1))
                    p0 = (pi % OG) * PT
                    nc.vector.tensor_copy(outst[:, :, p0:p0 + PT], op)
                    if pi % OG == OG - 1:
                        og0 = (pi // OG) * OG * PT
                        for mc in range(2):
                            nc.sync.dma_start(
                                out=out_d[mc * 128:(mc + 1) * 128,
                                          og0:og0 + OG * PT],
                                in_=outst[:, mc, :])
                return run

            epilogue[0] = make_epilogue()
            if pi == NPAIRS - 1:
                epilogue[0]()
                epilogue[0] = None

            if rot_next is not None:
                rot_cur = rot_next


_NC_CACHE = {}


def _get_nc():
    if 'nc' not in _NC_CACHE:
        _NC_CACHE['nc'] = build_nc(sim_safe=False)
    return _NC_CACHE['nc']


def _run(inputs, trace=False):
    from concourse.bass_utils import run_bass_kernel_spmd
    nc = _get_nc()
    in_maps = host_prep(inputs)
    res = run_bass_kernel_spmd(nc, in_maps, core_ids=list(range(N_CORES)),
                               trace=trace)
    return gather_out(res.results), res


def kernel(**inputs):
    out, _ = _run(inputs, trace=False)
    return out
